# revision 7
# baseline (speedup 1.0000x reference)
# DCNv2 (modulated deformable conv) Trainium2 Bass kernel.
#
# Sharding: pure data parallel over 8 cores; core = (batch, H-half), each
# core computes a (256, 32, 64) output slab from a zero-padded input slab.
#
# Per-core pipeline:
#   1. offset/mask 3x3 conv on the PE (bf16 matmuls, fp32 PSUM, fused
#      bias (+tap/pad constants) and sigmoid on the ACT engine)
#   2. transpose the conv outputs to hw-on-partitions [128, NT*KK] and
#      compute all sampling fields (floor/frac/bilinear weights incl.
#      mask) there at full DVE lane occupancy; weights stored bf16
#   3. per 128-position tile: indices bounce through DRAM into the
#      16-partition wrapped layout, then one SWDGE dma_gather pulls
#      2KB rows (all 4 bilinear corners x 2 c-groups) from an
#      HBM-resident corner-packed copy of x
#   4. 4-corner weighted combine via all-bf16 scalar_tensor_tensor
#      chains (per-partition scalars = bilinear weights)
#   5. PE-transpose to c-on-partitions (batched ACT copy-out), then the
#      main contraction out[o,hw] = sum_{c,p} w[o,c,p] cols[c,p,hw] as
#      bf16 matmuls accumulating in PSUM; bias on the ACT copy.
# Everything is in plain hw order (no sigma permutation): the DRAM
# bounce's replicated read performs the wrap permutation directly.
import numpy as np
import ml_dtypes

import concourse.bass as bass
import concourse.mybir as mybir
from concourse import bacc
import concourse.tile as tile
from concourse import library_config
from concourse.masks import make_identity
from concourse import bass_utils

BF16 = ml_dtypes.bfloat16

B, C, H, W = 4, 256, 64, 64
O, K = 256, 3
KK = K * K
NCORES = 8
HH = H // 2            # 32 output rows per core
PAD = 5                # zero halo; requires |offset| < PAD - 1
HP, WP = 48, 80        # padded local input dims
L = HP * WP            # 3840 source locations
NHW = HH * W           # 2048 output positions per core
NT = NHW // 128        # 16 gather tiles
CG = C // 128
OG = O // 128
A = mybir.AluOpType
ACTF = mybir.ActivationFunctionType
FP32 = mybir.dt.float32
BF = mybir.dt.bfloat16
I16 = mybir.dt.int16

MAGIC = float(np.float32(2 ** 23))


def build_nc():
    nc = bacc.Bacc("TRN2", target_bir_lowering=False, num_devices=NCORES)

    x_cm_d = nc.dram_tensor("x_cm", [CG, 128, HP, WP], BF, kind="ExternalInput").ap()
    xTp_d = nc.dram_tensor("xTp", [L, 4 * C], BF, kind="ExternalInput").ap()
    w_om_d = nc.dram_tensor("w_om", [128, KK, CG, 73], BF, kind="ExternalInput").ap()
    b_om_d = nc.dram_tensor("b_om", [73, 1], FP32, kind="ExternalInput").ap()
    w_mm_d = nc.dram_tensor("w_mm", [128, KK, CG, OG, 128], BF,
                            kind="ExternalInput").ap()
    b_o_d = nc.dram_tensor("b_o", [128, OG, 1], FP32, kind="ExternalInput").ap()
    hioT_d = nc.dram_tensor("hioT", [128, NT, KK], FP32, kind="ExternalInput").ap()
    wioT_d = nc.dram_tensor("wioT", [128, NT, KK], FP32, kind="ExternalInput").ap()
    y_d = nc.dram_tensor("y", [OG, 128, NHW], FP32, kind="ExternalOutput").ap()

    with tile.TileContext(nc) as tc:
        with (
            tc.tile_pool(name="const", bufs=1) as const,
            tc.tile_pool(name="persist", bufs=1) as persist,
            tc.tile_pool(name="dramp", bufs=3, space="DRAM") as dramp,
            tc.tile_pool(name="ps_conv", bufs=1, space="PSUM") as ps_conv,
            tc.tile_pool(name="ps_ft", bufs=2, space="PSUM") as ps_ft,
            tc.tile_pool(name="ps_t", bufs=1, space="PSUM") as ps_t,
            tc.tile_pool(name="ps_m", bufs=2, space="PSUM") as ps_m,
        ):
            # ---- constants into SBUF ----
            w_om = const.tile([128, KK, CG, 73], BF)
            nc.sync.dma_start(w_om[:], w_om_d)
            w_mm = const.tile([128, KK, CG, OG, 128], BF)
            nc.sync.dma_start(w_mm[:], w_mm_d)
            b_om = const.tile([73, 1], FP32)
            nc.sync.dma_start(b_om[:], b_om_d)
            b_o = const.tile([128, OG, 1], FP32)
            nc.sync.dma_start(b_o[:], b_o_d)
            hioT = const.tile([128, NT, KK], FP32)
            nc.sync.dma_start(hioT[:], hioT_d)
            wioT = const.tile([128, NT, KK], FP32)
            nc.sync.dma_start(wioT[:], wioT_d)
            id32 = const.tile([32, 32], FP32)
            make_identity(nc, id32[:])
            idbf = const.tile([128, 128], BF)
            make_identity(nc, idbf[:])
            zeroc = const.tile([128, C], BF)
            nc.gpsimd.memset(zeroc[:], 0.0)
            nc.gpsimd.load_library(library_config.mlp)

            # bilinear weight fields + gather indices live through the loop
            w00T = persist.tile([128, NT, KK], BF)
            w01T = persist.tile([128, NT, KK], BF)
            w10T = persist.tile([128, NT, KK], BF)
            w11T = persist.tile([128, NT, KK], BF)
            idxT16 = persist.tile([128, NT, KK], I16)

            fld_cm = tc.tile_pool(name="fld", bufs=1)
            fld = fld_cm.__enter__()
            x_sb = []
            for cg in range(CG):
                xt = fld.tile([128, HP, WP], BF, name=f"xsb{cg}")
                nc.sync.dma_start(xt[:], x_cm_d[cg])
                x_sb.append(xt)

            # ---- offset/mask conv ----
            # psum channel layout: [0:9] off_y, [32:41] off_x, [64:73] mask
            # (engine APs may only start at partitions 0/32/64/96); each is
            # copied to its own base-0 tile (TensorTensor requires equal
            # base partitions for SBUF operands)
            offy_s = fld.tile([KK, NHW], FP32)
            offx_s = fld.tile([KK, NHW], FP32)
            msk = fld.tile([KK, NHW], FP32)
            for blk in range(8):
                # matmul rhs must be one contiguous free dim: stream 4 full
                # padded rows (N=320) and discard the pad columns on copy-out
                ps = ps_conv.tile([73, 4 * WP], FP32, tag="psc")
                r0 = blk * 4
                n = 0
                for cg in range(CG):
                    for tap in range(KK):
                        ky, kx = tap // K, tap % K
                        rhs = x_sb[cg][:, r0 + 4 + ky, 4 + kx:]
                        rhs = bass.AP(tensor=rhs.tensor, offset=rhs.offset,
                                      ap=[rhs.ap[0], [1, 4 * WP]])
                        nc.tensor.matmul(ps[:], w_om[:, tap, cg], rhs,
                                         start=(n == 0), stop=(n == 2 * KK - 1))
                        n += 1
                sl = slice(blk * 4 * W, (blk + 1) * 4 * W)
                psv = [None, None, None]
                for i, base in enumerate((0, 32, 64)):
                    p4 = ps[base:base + 9].rearrange("c (r x) -> c r x", r=4)
                    psv[i] = p4[:, :, 0:W]
                nc.scalar.activation(offy_s[:, sl], psv[0], ACTF.Identity,
                                     bias=b_om[0:9])
                nc.scalar.activation(offx_s[:, sl], psv[1], ACTF.Identity,
                                     bias=b_om[32:41])
                nc.scalar.activation(msk[:, sl], psv[2], ACTF.Sigmoid,
                                     bias=b_om[64:73])

            # ---- transpose conv outputs to hw-on-partitions [128, NT, KK] ----
            offyT = fld.tile([128, NT, KK], FP32)
            offxT = fld.tile([128, NT, KK], FP32)
            mskT = fld.tile([128, NT, KK], FP32)
            for t in range(NT):
                for src, dstT in ((offy_s, offyT), (offx_s, offxT),
                                  (msk, mskT)):
                    psf = ps_ft.tile([128, KK], FP32, tag="psf", name="psf")
                    nc.tensor.transpose(psf[:], src[0:KK, t * 128:(t + 1) * 128],
                                        id32[0:KK, 0:KK])
                    nc.scalar.activation(dstT[:, t], psf[:], ACTF.Identity)

            # ---- sampling fields at full lane occupancy [128, NT*KK] ----
            py = fld.tile([128, NT, KK], FP32)
            px = fld.tile([128, NT, KK], FP32)
            nc.vector.tensor_tensor(py[:], offyT[:], hioT[:], A.add)
            nc.vector.tensor_tensor(px[:], offxT[:], wioT[:], A.add)

            def floor_clamp(dst, src, hi):
                # dst = clamp(floor(src), 0, hi) via magic-number round(src-0.5)
                t1 = fld.tile([128, NT, KK], FP32, tag="fc1", name="fc1")
                nc.vector.tensor_scalar(t1[:], src[:], MAGIC - 0.5, None, A.add)
                nc.vector.tensor_scalar(t1[:], t1[:], MAGIC, None, A.subtract)
                nc.vector.tensor_scalar(dst[:], t1[:], 0.0, float(hi),
                                        A.max, A.min)

            y0 = fld.tile([128, NT, KK], FP32)
            x0 = fld.tile([128, NT, KK], FP32)
            floor_clamp(y0, py, HP - 2)
            floor_clamp(x0, px, WP - 2)
            fy = fld.tile([128, NT, KK], FP32)
            fx = fld.tile([128, NT, KK], FP32)
            nc.vector.tensor_tensor(fy[:], py[:], y0[:], A.subtract)
            nc.vector.tensor_tensor(fx[:], px[:], x0[:], A.subtract)

            u = fld.tile([128, NT, KK], FP32)
            gy = fld.tile([128, NT, KK], FP32)
            nc.vector.tensor_tensor(u[:], fy[:], mskT[:], A.mult)
            nc.vector.tensor_tensor(gy[:], mskT[:], u[:], A.subtract)
            nc.vector.tensor_tensor(w01T[:], gy[:], fx[:], A.mult)
            nc.vector.tensor_tensor(w00T[:], gy[:], w01T[:], A.subtract)
            nc.vector.tensor_tensor(w11T[:], u[:], fx[:], A.mult)
            nc.vector.tensor_tensor(w10T[:], u[:], w11T[:], A.subtract)

            idxf = fld.tile([128, NT, KK], FP32)
            nc.vector.scalar_tensor_tensor(idxf[:], y0[:], float(WP), x0[:],
                                           A.mult, A.add)
            nc.vector.tensor_copy(idxT16[:], idxf[:])

            fld_cm.__exit__(None, None, None)

            # ---- per-tile gather + combine + matmul ----
            qp_cm = tc.tile_pool(name="qp", bufs=3)
            qp = qp_cm.__enter__()
            accp_cm = tc.tile_pool(name="accp", bufs=2)
            accp = accp_cm.__enter__()
            colsTp_cm = tc.tile_pool(name="colsTp", bufs=3)
            colsTp = colsTp_cm.__enter__()
            colsp_cm = tc.tile_pool(name="colsp", bufs=2)
            colsp = colsp_cm.__enter__()
            wrapp_cm = tc.tile_pool(name="wrapp", bufs=3)
            wrapp = wrapp_cm.__enter__()
            outp_cm = tc.tile_pool(name="outp", bufs=2)
            outp = outp_cm.__enter__()
            cols_sb = None
            for t in range(NT):
                # idx -> DRAM bounce; the replicated read performs the wrap
                # permutation wrapped[p][8*tap+r] = idx[tap, 16*r+p] needed by
                # the gather engine for plain hw order out[P] = pos t*128+P
                db = dramp.tile([16, 72], I16, tag="db", name="db")
                dst_ap = bass.AP(tensor=db.tensor, offset=db.offset,
                                 ap=[[1, 8], [72, 16], [8, 9]])
                nc.sync.dma_start(dst_ap, idxT16[:, t])
                wrapped = wrapp.tile([128, 72], I16, tag="wr", name="wr")
                rep_ap = bass.AP(tensor=db.tensor, offset=db.offset,
                                 ap=[[0, 8], [72, 16], [1, 72]])
                nc.sync.dma_start(wrapped[:], rep_ap)

                # gather Q[128(hw), 9(tap), 2KB(v00|v01|v10|v11 channels)]
                Q = qp.tile([128, KK, 4 * C], BF, tag="Q", name="Q")
                nc.gpsimd.dma_gather(
                    out_ap=Q[:], in_ap=xTp_d, idxs_ap=wrapped[:],
                    num_idxs=KK * 128, num_idxs_reg=KK * 128,
                    elem_size=4 * C, single_packet=False,
                )

                # weighted 4-corner combine -> colsT [128(hw), KK, C] bf16
                colsT = colsTp.tile([128, KK, C], BF, tag="colsT", name="colsT")
                acc = accp.tile([128, C], BF, tag="acc", name="acc")
                for tap in range(KK):
                    nc.vector.scalar_tensor_tensor(
                        acc[:], Q[:, tap, 0:C], w00T[:, t, tap:tap + 1],
                        zeroc[:], A.mult, A.add)
                    nc.vector.scalar_tensor_tensor(
                        acc[:], Q[:, tap, C:2 * C], w01T[:, t, tap:tap + 1],
                        acc[:], A.mult, A.add)
                    nc.vector.scalar_tensor_tensor(
                        acc[:], Q[:, tap, 2 * C:3 * C], w10T[:, t, tap:tap + 1],
                        acc[:], A.mult, A.add)
                    nc.vector.scalar_tensor_tensor(
                        colsT[:, tap], Q[:, tap, 3 * C:4 * C],
                        w11T[:, t, tap:tap + 1], acc[:], A.mult, A.add)

                # transpose to cols [128(c), KK, CG, 512(hw)] bf16; batch 4
                # [128,128] PSUM chunks per ACT copy-out
                if t % 4 == 0:
                    cols_sb = colsp.tile([128, KK, CG, 512], BF, tag="cols",
                                         name="cols")
                for tap in range(KK):
                    for cg in range(CG):
                        pst = ps_t.tile([128, 128], BF, tag="pst", name="pst",
                                        bufs=2)
                        nc.tensor.transpose(
                            pst[:], colsT[:, tap, cg * 128:(cg + 1) * 128],
                            idbf[:])
                        nc.scalar.activation(
                            cols_sb[:, tap, cg,
                                    (t % 4) * 128:(t % 4 + 1) * 128],
                            pst[:], ACTF.Identity)

                # main contraction per 4-tile group
                if t % 4 == 3:
                    g = t // 4
                    for og in range(OG):
                        psO = ps_m.tile([128, 512], FP32, tag="psO", name="psO")
                        n = 0
                        for cg in range(CG):
                            for tap in range(KK):
                                nc.tensor.matmul(
                                    psO[:], w_mm[:, tap, cg, og],
                                    cols_sb[:, tap, cg],
                                    start=(n == 0), stop=(n == 2 * KK - 1))
                                n += 1
                        out_t = outp.tile([128, 512], FP32, tag="out",
                                          name="out_t")
                        nc.scalar.activation(out_t[:], psO[:], ACTF.Identity,
                                             bias=b_o[:, og])
                        nc.sync.dma_start(y_d[og, :, g * 512:(g + 1) * 512],
                                          out_t[:])
            for p in (outp_cm, wrapp_cm, colsp_cm, colsTp_cm, accp_cm, qp_cm):
                p.__exit__(None, None, None)
    nc.compile()
    return nc


# ---------------- host side ----------------

def host_prep(x, w_off, b_off, w_mask, b_mask, w_dcn, b_dcn):
    """Build the 8 per-core input maps (pure layout prep / sharding)."""
    x = np.asarray(x, np.float32)
    w_off = np.asarray(w_off, np.float32)
    w_mask = np.asarray(w_mask, np.float32)
    b_off = np.asarray(b_off, np.float32)
    b_mask = np.asarray(b_mask, np.float32)
    w_dcn = np.asarray(w_dcn, np.float32)
    b_dcn = np.asarray(b_dcn, np.float32)

    wcat = np.zeros((73, C, K, K), np.float32)
    wcat[0:9] = w_off[0::2]
    wcat[32:41] = w_off[1::2]
    wcat[64:73] = w_mask
    w_om = np.zeros((128, KK, CG, 73), BF16)
    for tap in range(KK):
        ky, kx = tap // K, tap % K
        for cg in range(CG):
            w_om[:, tap, cg] = (
                wcat[:, cg * 128:(cg + 1) * 128, ky, kx].T.astype(BF16))

    ky_t = np.repeat(np.arange(K), K).astype(np.float32)
    kx_t = np.tile(np.arange(K), K).astype(np.float32)
    b_om = np.zeros((73, 1), np.float32)
    b_om[0:9, 0] = b_off[0::2] + ky_t - 1 + PAD
    b_om[32:41, 0] = b_off[1::2] + kx_t - 1 + PAD
    b_om[64:73, 0] = b_mask

    w_mm = np.zeros((128, KK, CG, OG, 128), BF16)
    for tap in range(KK):
        ky, kx = tap // K, tap % K
        for cg in range(CG):
            for og in range(OG):
                w_mm[:, tap, cg, og] = w_dcn[
                    og * 128:(og + 1) * 128, cg * 128:(cg + 1) * 128,
                    ky, kx].T.astype(BF16)
    b_o = b_dcn.reshape(OG, 128, 1).transpose(1, 0, 2).copy()

    hw = np.arange(NHW)
    rowg = (hw // W).astype(np.float32).reshape(NT, 128).T      # [128, NT]
    colg = (hw % W).astype(np.float32).reshape(NT, 128).T
    hioT = np.ascontiguousarray(
        np.broadcast_to(rowg[:, :, None], (128, NT, KK)), np.float32)
    wioT = np.ascontiguousarray(
        np.broadcast_to(colg[:, :, None], (128, NT, KK)), np.float32)

    shared = dict(w_om=w_om, b_om=b_om, w_mm=w_mm, b_o=b_o,
                  hioT=hioT, wioT=wioT)

    in_maps = []
    for core in range(NCORES):
        b, half = core // 2, core % 2
        h0 = half * HH
        xp = np.zeros((C, HP, WP), np.float32)
        glo, ghi = h0 - PAD, h0 + HH + PAD
        slo, shi = max(glo, 0), min(ghi, H)
        xp[:, slo - glo: slo - glo + (shi - slo), PAD:PAD + W] = x[b, :, slo:shi, :]
        xbf = xp.astype(BF16)
        x_cm = np.ascontiguousarray(xbf.reshape(CG, 128, HP, WP))
        xf = xbf.reshape(C, L)
        # corner-packed gather rows: row l = [x[l], x[l+1], x[l+WP], x[l+WP+1]]
        xfp = np.zeros((C, L + WP + 1), BF16)
        xfp[:, :L] = xf
        xT = xfp.T
        xTp = np.empty((L, 4 * C), BF16)
        xTp[:, 0:C] = xT[0:L]
        xTp[:, C:2 * C] = xT[1:L + 1]
        xTp[:, 2 * C:3 * C] = xT[WP:L + WP]
        xTp[:, 3 * C:4 * C] = xT[WP + 1:L + WP + 1]
        im = dict(shared)
        im["x_cm"] = x_cm
        im["xTp"] = xTp
        in_maps.append(im)
    return in_maps


_NC_CACHE = {}


def kernel(**inputs):
    if "nc" not in _NC_CACHE:
        _NC_CACHE["nc"] = build_nc()
    nc = _NC_CACHE["nc"]
    in_maps = host_prep(**inputs)
    res = bass_utils.run_bass_kernel_spmd(nc, in_maps,
                                          core_ids=list(range(NCORES)))
    out = np.zeros((B, O, H, W), np.float32)
    for core in range(NCORES):
        b, half = core // 2, core % 2
        yv = np.asarray(res.results[core]["y"], np.float32).reshape(O, HH, W)
        out[b, :, half * HH:(half + 1) * HH, :] = yv
    return out


# revision 23
# speedup vs baseline: 1.4643x; 1.4643x over previous
# DCNv2 (modulated deformable conv) Trainium2 Bass kernel.
#
# Sharding: pure data parallel over 8 cores; core = (batch, H-half), each
# core computes a (256, 32, 64) output slab from a zero-padded input slab.
#
# Per-core pipeline:
#   1. offset/mask 3x3 conv on the PE (bf16 matmuls, fp32 PSUM, fused
#      bias (+tap/pad constants) and sigmoid on the ACT engine)
#   2. transpose the conv outputs to hw-on-partitions [128, NT*KK] and
#      compute all sampling fields (floor/frac/bilinear weights incl.
#      mask) there at full DVE lane occupancy; weights stored bf16
#   3. per 128-position tile: indices bounce through DRAM into the
#      16-partition wrapped layout, then one SWDGE dma_gather pulls
#      2KB rows (all 4 bilinear corners x 2 c-groups) from an
#      HBM-resident corner-packed copy of x
#   4. 4-corner weighted combine via all-bf16 scalar_tensor_tensor
#      chains (per-partition scalars = bilinear weights)
#   5. PE-transpose to c-on-partitions (batched ACT copy-out), then the
#      main contraction out[o,hw] = sum_{c,p} w[o,c,p] cols[c,p,hw] as
#      bf16 matmuls accumulating in PSUM; bias on the ACT copy.
# Everything is in plain hw order (no sigma permutation): the DRAM
# bounce's replicated read performs the wrap permutation directly.
import numpy as np
import ml_dtypes

import concourse.bass as bass
import concourse.mybir as mybir
from concourse import bacc
import concourse.tile as tile
from concourse import library_config
from concourse.masks import make_identity
from concourse import bass_utils

BF16 = ml_dtypes.bfloat16

B, C, H, W = 4, 256, 64, 64
O, K = 256, 3
KK = K * K
NCORES = 8
HH = H // 2            # 32 output rows per core
PAD = 5                # zero halo; requires |offset| < PAD - 1
HP, WP = 48, 80        # padded local input dims
L = HP * WP            # 3840 source locations
NHW = HH * W           # 2048 output positions per core
NT = NHW // 128        # 16 gather tiles
CG = C // 128
OG = O // 128
A = mybir.AluOpType
ACTF = mybir.ActivationFunctionType
FP32 = mybir.dt.float32
BF = mybir.dt.bfloat16
I16 = mybir.dt.int16

MAGIC = float(np.float32(2 ** 23))


def build_nc():
    nc = bacc.Bacc("TRN2", target_bir_lowering=False, num_devices=NCORES)

    x_cm_d = nc.dram_tensor("x_cm", [CG, 128, HP, WP], BF, kind="ExternalInput").ap()
    xTp_d = nc.dram_tensor("xTp", [L, 4 * C], BF, kind="ExternalInput").ap()
    w_om_d = nc.dram_tensor("w_om", [128, KK, CG, 73], BF, kind="ExternalInput").ap()
    b_om_d = nc.dram_tensor("b_om", [73, 1], FP32, kind="ExternalInput").ap()
    w_mm_d = nc.dram_tensor("w_mm", [128, KK, CG, OG, 128], BF,
                            kind="ExternalInput").ap()
    b_o_d = nc.dram_tensor("b_o", [128, OG, 1], FP32, kind="ExternalInput").ap()
    hioT_d = nc.dram_tensor("hioT", [128, NT, KK], FP32, kind="ExternalInput").ap()
    wioT_d = nc.dram_tensor("wioT", [128, NT, KK], FP32, kind="ExternalInput").ap()
    prm_d = nc.dram_tensor("prm", [128, 128], FP32, kind="ExternalInput").ap()
    y_d = nc.dram_tensor("y", [OG, 128, NHW], FP32, kind="ExternalOutput").ap()

    with tile.TileContext(nc) as tc:
        with (
            tc.tile_pool(name="const", bufs=1) as const,
            tc.tile_pool(name="persist", bufs=1) as persist,
            tc.tile_pool(name="dramp", bufs=3, space="DRAM") as dramp,
            tc.tile_pool(name="ps_conv", bufs=1, space="PSUM") as ps_conv,
            tc.tile_pool(name="ps_ft", bufs=2, space="PSUM") as ps_ft,
            tc.tile_pool(name="ps_t", bufs=1, space="PSUM") as ps_t,
            tc.tile_pool(name="ps_m", bufs=2, space="PSUM") as ps_m,
        ):
            # ---- constants into SBUF ----
            w_om = const.tile([128, KK, CG, 73], BF)
            nc.sync.dma_start(w_om[:], w_om_d)
            w_mm = const.tile([128, KK, CG, OG, 128], BF)
            nc.sync.dma_start(w_mm[:], w_mm_d)
            b_om = const.tile([73, 1], FP32)
            nc.sync.dma_start(b_om[:], b_om_d)
            b_o = const.tile([128, OG, 1], FP32)
            nc.sync.dma_start(b_o[:], b_o_d)
            hioT = const.tile([128, NT, KK], FP32)
            nc.sync.dma_start(hioT[:], hioT_d)
            wioT = const.tile([128, NT, KK], FP32)
            nc.sync.dma_start(wioT[:], wioT_d)
            id32 = const.tile([32, 32], FP32)
            make_identity(nc, id32[:])
            prm = const.tile([128, 128], FP32)
            nc.sync.dma_start(prm[:], prm_d)
            idbf = const.tile([128, 128], BF)
            make_identity(nc, idbf[:])
            zeroc = const.tile([128, C], BF)
            nc.gpsimd.memset(zeroc[:], 0.0)
            nc.gpsimd.load_library(library_config.mlp)

            # bilinear weight fields + gather indices live through the loop
            w00T = persist.tile([128, NT, KK], BF)
            w01T = persist.tile([128, NT, KK], BF)
            w10T = persist.tile([128, NT, KK], BF)
            w11T = persist.tile([128, NT, KK], BF)
            idx16h = persist.tile([KK, NHW], I16)

            fld_cm = tc.tile_pool(name="fld", bufs=1)
            fld = fld_cm.__enter__()
            x_sb = []
            for cg in range(CG):
                xt = fld.tile([128, HP, WP], BF, name=f"xsb{cg}")
                nc.sync.dma_start(xt[:], x_cm_d[cg])
                x_sb.append(xt)

            # ---- offset/mask conv ----
            # psum channel layout: [0:9] off_y, [32:41] off_x, [64:73] mask
            # (engine APs may only start at partitions 0/32/64/96); each is
            # copied to its own base-0 tile (TensorTensor requires equal
            # base partitions for SBUF operands)
            offy_s = fld.tile([KK, NHW], FP32)
            offx_s = fld.tile([KK, NHW], FP32)
            msk = fld.tile([KK, NHW], FP32)
            for blk in range(8):
                # matmul rhs must be one contiguous free dim: stream 4 full
                # padded rows (N=320) and discard the pad columns on copy-out
                ps = ps_conv.tile([73, 4 * WP], FP32, tag="psc")
                r0 = blk * 4
                n = 0
                for cg in range(CG):
                    for tap in range(KK):
                        ky, kx = tap // K, tap % K
                        rhs = x_sb[cg][:, r0 + 4 + ky, 4 + kx:]
                        rhs = bass.AP(tensor=rhs.tensor, offset=rhs.offset,
                                      ap=[rhs.ap[0], [1, 4 * WP]])
                        nc.tensor.matmul(ps[:], w_om[:, tap, cg], rhs,
                                         start=(n == 0), stop=(n == 2 * KK - 1))
                        n += 1
                sl = slice(blk * 4 * W, (blk + 1) * 4 * W)
                psv = [None, None, None]
                for i, base in enumerate((0, 32, 64)):
                    p4 = ps[base:base + 9].rearrange("c (r x) -> c r x", r=4)
                    psv[i] = p4[:, :, 0:W]
                nc.scalar.activation(offy_s[:, sl], psv[0], ACTF.Identity,
                                     bias=b_om[0:9])
                nc.scalar.activation(offx_s[:, sl], psv[1], ACTF.Identity,
                                     bias=b_om[32:41])
                nc.scalar.activation(msk[:, sl], psv[2], ACTF.Sigmoid,
                                     bias=b_om[64:73])

            # ---- transpose conv outputs to hw-on-partitions [128, NT, KK] ----
            offyT = fld.tile([128, NT, KK], FP32)
            offxT = fld.tile([128, NT, KK], FP32)
            mskT = fld.tile([128, NT, KK], FP32)
            for t in range(NT):
                for src, dstT in ((offy_s, offyT), (offx_s, offxT),
                                  (msk, mskT)):
                    psf = ps_ft.tile([128, KK], FP32, tag="psf", name="psf")
                    nc.tensor.transpose(psf[:], src[0:KK, t * 128:(t + 1) * 128],
                                        id32[0:KK, 0:KK])
                    nc.scalar.activation(dstT[:, t], psf[:], ACTF.Identity)

            # ---- sampling fields at full lane occupancy [128, NT*KK] ----
            py = fld.tile([128, NT, KK], FP32)
            px = fld.tile([128, NT, KK], FP32)
            nc.vector.tensor_tensor(py[:], offyT[:], hioT[:], A.add)
            nc.vector.tensor_tensor(px[:], offxT[:], wioT[:], A.add)

            def floor_clamp(dst, src, hi):
                # dst = clamp(floor(src), 0, hi) via magic-number round(src-0.5)
                t1 = fld.tile([128, NT, KK], FP32, tag="fc1", name="fc1")
                nc.vector.tensor_scalar(t1[:], src[:], MAGIC - 0.5, None, A.add)
                nc.vector.tensor_scalar(t1[:], t1[:], MAGIC, None, A.subtract)
                nc.vector.tensor_scalar(dst[:], t1[:], 0.0, float(hi),
                                        A.max, A.min)

            y0 = fld.tile([128, NT, KK], FP32)
            x0 = fld.tile([128, NT, KK], FP32)
            floor_clamp(y0, py, HP - 2)
            floor_clamp(x0, px, WP - 2)
            fy = fld.tile([128, NT, KK], FP32)
            fx = fld.tile([128, NT, KK], FP32)
            nc.vector.tensor_tensor(fy[:], py[:], y0[:], A.subtract)
            nc.vector.tensor_tensor(fx[:], px[:], x0[:], A.subtract)

            u = fld.tile([128, NT, KK], FP32)
            gy = fld.tile([128, NT, KK], FP32)
            nc.vector.tensor_tensor(u[:], fy[:], mskT[:], A.mult)
            nc.vector.tensor_tensor(gy[:], mskT[:], u[:], A.subtract)
            nc.vector.tensor_tensor(w01T[:], gy[:], fx[:], A.mult)
            nc.vector.tensor_tensor(w00T[:], gy[:], w01T[:], A.subtract)
            nc.vector.tensor_tensor(w11T[:], u[:], fx[:], A.mult)
            nc.vector.tensor_tensor(w10T[:], u[:], w11T[:], A.subtract)

            idxf = fld.tile([128, NT, KK], FP32)
            nc.vector.scalar_tensor_tensor(idxf[:], y0[:], float(WP), x0[:],
                                           A.mult, A.add)
            # un-transpose idx back to hw-major [KK, NHW] via exact fp32
            # matmul against the host-built permutation prm[P, j] =
            # 1[P == 16*(j%8) + j//8], then cast to i16.  The permuted column
            # order makes the per-tile idx DRAM bounce writes contiguous.
            for t in range(NT):
                psI = ps_ft.tile([KK, 128], FP32, tag="psI", name="psI",
                                 bufs=1)
                nc.tensor.matmul(psI[:], idxf[:, t], prm[:], start=True,
                                 stop=True)
                nc.vector.tensor_copy(idx16h[:, t * 128:(t + 1) * 128], psI[:])

            fld_cm.__exit__(None, None, None)

            # ---- per-tile gather + combine + matmul ----
            qp_cm = tc.tile_pool(name="qp", bufs=3)
            qp = qp_cm.__enter__()
            accp_cm = tc.tile_pool(name="accp", bufs=2)
            accp = accp_cm.__enter__()
            colsTp_cm = tc.tile_pool(name="colsTp", bufs=3)
            colsTp = colsTp_cm.__enter__()
            colsp_cm = tc.tile_pool(name="colsp", bufs=2)
            colsp = colsp_cm.__enter__()
            wrapp_cm = tc.tile_pool(name="wrapp", bufs=3)
            wrapp = wrapp_cm.__enter__()
            outp_cm = tc.tile_pool(name="outp", bufs=2)
            outp = outp_cm.__enter__()
            cols_sb = None
            for t in range(NT):
                # idx -> DRAM bounce (wrapped [16,72] layout, 16B runs) ->
                # replicated read; sigma makes both DMAs contiguous
                db = dramp.tile([16, 72], I16, tag="db", name="db")
                dst_ap = bass.AP(tensor=db.tensor, offset=db.offset,
                                 ap=[[8, KK], [72, 16], [1, 8]])
                src_w = idx16h[:, t * 128:(t + 1) * 128].rearrange(
                    "b (p q) -> b p q", p=16)
                nc.sync.dma_start(dst_ap, src_w)
                wrapped = wrapp.tile([128, 72], I16, tag="wr", name="wr")
                rep_ap = bass.AP(tensor=db.tensor, offset=db.offset,
                                 ap=[[0, 8], [72, 16], [1, 72]])
                nc.sync.dma_start(wrapped[:], rep_ap)

                # gather Q[128(sig hw), 9(tap), 2KB(v00|v01|v10|v11 channels)]
                Q = qp.tile([128, KK, 4 * C], BF, tag="Q", name="Q")
                nc.gpsimd.dma_gather(
                    out_ap=Q[:], in_ap=xTp_d, idxs_ap=wrapped[:],
                    num_idxs=KK * 128, num_idxs_reg=KK * 128,
                    elem_size=4 * C, single_packet=False,
                )

                # weighted 4-corner combine -> colsT [128(hw), KK, C] bf16
                colsT = colsTp.tile([128, KK, C], BF, tag="colsT", name="colsT")
                acc = accp.tile([128, C], BF, tag="acc", name="acc")
                for tap in range(KK):
                    nc.vector.scalar_tensor_tensor(
                        acc[:], Q[:, tap, 0:C], w00T[:, t, tap:tap + 1],
                        zeroc[:], A.mult, A.add)
                    nc.vector.scalar_tensor_tensor(
                        acc[:], Q[:, tap, C:2 * C], w01T[:, t, tap:tap + 1],
                        acc[:], A.mult, A.add)
                    nc.vector.scalar_tensor_tensor(
                        acc[:], Q[:, tap, 2 * C:3 * C], w10T[:, t, tap:tap + 1],
                        acc[:], A.mult, A.add)
                    nc.vector.scalar_tensor_tensor(
                        colsT[:, tap], Q[:, tap, 3 * C:4 * C],
                        w11T[:, t, tap:tap + 1], acc[:], A.mult, A.add)

                # transpose to cols [128(c), KK, CG, 512(hw)] bf16; batch 4
                # [128,128] PSUM chunks per ACT copy-out
                if t % 4 == 0:
                    cols_sb = colsp.tile([128, KK, CG, 512], BF, tag="cols",
                                         name="cols")
                for tap in range(KK):
                    for cg in range(CG):
                        pst = ps_t.tile([128, 128], BF, tag="pst", name="pst",
                                        bufs=2)
                        nc.tensor.transpose(
                            pst[:], colsT[:, tap, cg * 128:(cg + 1) * 128],
                            idbf[:])
                        nc.scalar.activation(
                            cols_sb[:, tap, cg,
                                    (t % 4) * 128:(t % 4 + 1) * 128],
                            pst[:], ACTF.Identity)

                # main contraction per 4-tile group
                if t % 4 == 3:
                    g = t // 4
                    for og in range(OG):
                        psO = ps_m.tile([128, 512], FP32, tag="psO", name="psO")
                        n = 0
                        for cg in range(CG):
                            for tap in range(KK):
                                nc.tensor.matmul(
                                    psO[:], w_mm[:, tap, cg, og],
                                    cols_sb[:, tap, cg],
                                    start=(n == 0), stop=(n == 2 * KK - 1))
                                n += 1
                        out_t = outp.tile([128, 512], FP32, tag="out",
                                          name="out_t")
                        nc.scalar.activation(out_t[:], psO[:], ACTF.Identity,
                                             bias=b_o[:, og])
                        nc.sync.dma_start(y_d[og, :, g * 512:(g + 1) * 512],
                                          out_t[:])
            for p in (outp_cm, wrapp_cm, colsp_cm, colsTp_cm, accp_cm, qp_cm):
                p.__exit__(None, None, None)
    nc.compile()
    return nc


# ---------------- host side ----------------

def host_prep(x, w_off, b_off, w_mask, b_mask, w_dcn, b_dcn):
    """Build the 8 per-core input maps (pure layout prep / sharding)."""
    x = np.asarray(x, np.float32)
    w_off = np.asarray(w_off, np.float32)
    w_mask = np.asarray(w_mask, np.float32)
    b_off = np.asarray(b_off, np.float32)
    b_mask = np.asarray(b_mask, np.float32)
    w_dcn = np.asarray(w_dcn, np.float32)
    b_dcn = np.asarray(b_dcn, np.float32)

    wcat = np.zeros((73, C, K, K), np.float32)
    wcat[0:9] = w_off[0::2]
    wcat[32:41] = w_off[1::2]
    wcat[64:73] = w_mask
    w_om = np.zeros((128, KK, CG, 73), BF16)
    for tap in range(KK):
        ky, kx = tap // K, tap % K
        for cg in range(CG):
            w_om[:, tap, cg] = (
                wcat[:, cg * 128:(cg + 1) * 128, ky, kx].T.astype(BF16))

    ky_t = np.repeat(np.arange(K), K).astype(np.float32)
    kx_t = np.tile(np.arange(K), K).astype(np.float32)
    b_om = np.zeros((73, 1), np.float32)
    b_om[0:9, 0] = b_off[0::2] + ky_t - 1 + PAD
    b_om[32:41, 0] = b_off[1::2] + kx_t - 1 + PAD
    b_om[64:73, 0] = b_mask

    w_mm = np.zeros((128, KK, CG, OG, 128), BF16)
    for tap in range(KK):
        ky, kx = tap // K, tap % K
        for cg in range(CG):
            for og in range(OG):
                w_mm[:, tap, cg, og] = w_dcn[
                    og * 128:(og + 1) * 128, cg * 128:(cg + 1) * 128,
                    ky, kx].T.astype(BF16)
    b_o = b_dcn.reshape(OG, 128, 1).transpose(1, 0, 2).copy()

    hw = np.arange(NHW)
    rowg = (hw // W).astype(np.float32).reshape(NT, 128).T      # [128, NT]
    colg = (hw % W).astype(np.float32).reshape(NT, 128).T
    hioT = np.ascontiguousarray(
        np.broadcast_to(rowg[:, :, None], (128, NT, KK)), np.float32)
    wioT = np.ascontiguousarray(
        np.broadcast_to(colg[:, :, None], (128, NT, KK)), np.float32)
    j = np.arange(128)
    prm = np.zeros((128, 128), np.float32)
    prm[16 * (j % 8) + j // 8, j] = 1.0

    shared = dict(w_om=w_om, b_om=b_om, w_mm=w_mm, b_o=b_o,
                  hioT=hioT, wioT=wioT, prm=prm)

    in_maps = []
    for core in range(NCORES):
        b, half = core // 2, core % 2
        h0 = half * HH
        xp = np.zeros((C, HP, WP), np.float32)
        glo, ghi = h0 - PAD, h0 + HH + PAD
        slo, shi = max(glo, 0), min(ghi, H)
        xp[:, slo - glo: slo - glo + (shi - slo), PAD:PAD + W] = x[b, :, slo:shi, :]
        xbf = xp.astype(BF16)
        x_cm = np.ascontiguousarray(xbf.reshape(CG, 128, HP, WP))
        xf = xbf.reshape(C, L)
        # corner-packed gather rows: row l = [x[l], x[l+1], x[l+WP], x[l+WP+1]]
        xfp = np.zeros((C, L + WP + 1), BF16)
        xfp[:, :L] = xf
        xT = xfp.T
        xTp = np.empty((L, 4 * C), BF16)
        xTp[:, 0:C] = xT[0:L]
        xTp[:, C:2 * C] = xT[1:L + 1]
        xTp[:, 2 * C:3 * C] = xT[WP:L + WP]
        xTp[:, 3 * C:4 * C] = xT[WP + 1:L + WP + 1]
        im = dict(shared)
        im["x_cm"] = x_cm
        im["xTp"] = xTp
        in_maps.append(im)
    return in_maps


_NC_CACHE = {}


def kernel(**inputs):
    if "nc" not in _NC_CACHE:
        _NC_CACHE["nc"] = build_nc()
    nc = _NC_CACHE["nc"]
    in_maps = host_prep(**inputs)
    res = bass_utils.run_bass_kernel_spmd(nc, in_maps,
                                          core_ids=list(range(NCORES)))
    out = np.zeros((B, O, H, W), np.float32)
    for core in range(NCORES):
        b, half = core // 2, core % 2
        yv = np.asarray(res.results[core]["y"], np.float32).reshape(O, HH, W)
        out[b, :, half * HH:(half + 1) * HH, :] = yv
    return out
